# revision 1
# baseline (speedup 1.0000x reference)
"""Multi-head self-attention (RoPE + causal) Trainium2 Bass kernel.

Problem: b=2, s=2048, d_model=1024, 16 heads x 64 dims, causal, RoPE.
Sharding over 8 NeuronCores: core c -> (batch bi = c//4, head group g = c%4
of 4 heads). Each core computes its 4 heads' attention from x[bi] and
produces a partial output projection (Wo column-block); the host sums the
4 partials per batch element.

Per-core device layout (all matmul operands bf16, fp32 PSUM accumulate):
  xT     [1024, 2048]  x[bi] transposed (d_model on partitions)
  wqkv   [1024, 768]   [WqT | WkT | WvT] column blocks, Q/K rows permuted:
                       per pair-tile t: [h(2t) even dims, h(2t+1) even,
                       h(2t) odd, h(2t+1) odd] (32 rows each) so RoPE is
                       a half-swap + elementwise mul/add
  QT/KT  [128, 2048]x2 projected+roped, tile t holds heads 2t, 2t+1
  scores ST[k,q] via PE (contraction = head dims, 2x2 row-group packing)
  softmax: exp on ACT (scale=1/8 folded in), causal mask = multiplicative
           bf16 tiles, denominator = ones column appended to V (row 64 of
           the AV PSUM accumulator), divide via reciprocal + gpsimd
           partition_broadcast
  out    [2048, 1024] fp32 partial = O @ Wo_block
"""

import os
import sys
from contextlib import ExitStack

import numpy as np

for _p in ("/root/.axon_site", "/root/.axon_site/_ro/trn_rl_repo", "/opt/trn_rl_repo"):
    if os.path.isdir(_p) and _p not in sys.path:
        sys.path.append(_p)

import ml_dtypes  # noqa: E402
import concourse.bass as bass  # noqa: E402
import concourse.tile as tile  # noqa: E402
import concourse.mybir as mybir  # noqa: E402
from concourse import bacc  # noqa: E402
from concourse.bass import ts  # noqa: E402
from concourse.bass_utils import run_bass_kernel_spmd  # noqa: E402

BF16 = mybir.dt.bfloat16
F32 = mybir.dt.float32
NPBF16 = ml_dtypes.bfloat16

S = 2048
D = 1024
DK = 64
HEADS_PER_CORE = 4
THETA = 10000.0

_CACHE = {}


def _build_nc():
    nc = bacc.Bacc("TRN2", target_bir_lowering=False, debug=False, num_devices=8)
    xT = nc.dram_tensor("xT", [D, S], BF16, kind="ExternalInput").ap()
    wqkv = nc.dram_tensor("wqkv", [D, 768], BF16, kind="ExternalInput").ap()
    woT = nc.dram_tensor("woT", [256, D], BF16, kind="ExternalInput").ap()
    ropec = nc.dram_tensor("ropec", [128, 2048], BF16, kind="ExternalInput").ap()
    ropes = nc.dram_tensor("ropes", [128, 2048], BF16, kind="ExternalInput").ap()
    yp = nc.dram_tensor("yp", [S, D], BF16, kind="ExternalOutput").ap()

    Exp = mybir.ActivationFunctionType.Exp

    with ExitStack() as ctx:
        tc = ctx.enter_context(tile.TileContext(nc))
        const = ctx.enter_context(tc.tile_pool(name="const", bufs=1))
        sb = ctx.enter_context(tc.tile_pool(name="sb", bufs=2))
        expp = ctx.enter_context(tc.tile_pool(name="expp", bufs=6))
        outp = ctx.enter_context(tc.tile_pool(name="outp", bufs=3))
        psA = ctx.enter_context(tc.tile_pool(name="psA", bufs=2, space="PSUM"))
        psB = ctx.enter_context(tc.tile_pool(name="psB", bufs=4, space="PSUM"))

        # ---- persistent SBUF ----
        xts = [const.tile([128, S], BF16, tag=f"xt{i}", name=f"xt{i}") for i in range(8)]
        wts = [const.tile([128, 768], BF16, tag=f"wt{i}", name=f"wt{i}") for i in range(8)]
        wos = [const.tile([128, D], BF16, tag=f"wo{i}", name=f"wo{i}") for i in range(2)]
        ropec_sb = const.tile([128, 2048], BF16, tag="ropec")
        ropes_sb = const.tile([128, 2048], BF16, tag="ropes")
        v_sb = const.tile([128, 16 * 260 + 64], BF16, tag="v")
        ones65 = const.tile([65, 64], BF16, tag="ones65")
        qf = [const.tile([128, S], BF16, tag=f"qf{t}", name=f"qf{t}") for t in range(2)]
        kf = [const.tile([128, S], BF16, tag=f"kf{t}", name=f"kf{t}") for t in range(2)]
        ot = [const.tile([128, S], BF16, tag=f"ot{t}", name=f"ot{t}") for t in range(2)]

        # ---- input DMAs ----
        for i in range(8):
            nc.sync.dma_start(xts[i][:], xT[ts(i, 128), :])
            nc.sync.dma_start(wts[i][:], wqkv[ts(i, 128), :])
        for i in range(2):
            nc.sync.dma_start(wos[i][:], woT[ts(i, 128), :])
        nc.sync.dma_start(ropec_sb[:], ropec[:])
        nc.sync.dma_start(ropes_sb[:], ropes[:])

        # ones columns of v_sb: [128, 16 blocks, 4 heads, col 64]
        ones_ap = v_sb[:, 0 : 16 * 260].rearrange("p (b h x) -> p b h x", b=16, h=4)[
            :, :, :, 64:65
        ]
        nc.gpsimd.memset(ones_ap, 1.0)
        nc.gpsimd.memset(v_sb[:, 16 * 260 : 16 * 260 + 64], 0.0)
        nc.gpsimd.memset(ones65[:], 1.0)

        # ---- V projection: V[s, dv] natural layout ----
        for st in range(16):
            vp = psB.tile([128, 256], F32, tag="psb", name="vp")
            for kc in range(8):
                nc.tensor.matmul(
                    vp[:],
                    lhsT=xts[kc][:, ts(st, 128)],
                    rhs=wts[kc][:, 512:768],
                    start=(kc == 0),
                    stop=(kc == 7),
                )
            dst = v_sb[:, st * 260 : (st + 1) * 260].rearrange(
                "p (h x) -> p h x", h=4
            )[:, :, 0:64]
            vsrc = vp[:].rearrange("p (h x) -> p h x", h=4)
            nc.scalar.copy(dst, vsrc)

        # ---- Q/K projection + RoPE ----
        for c in range(4):
            for t in range(2):
                sp = psA.tile([128, 1024], F32, tag="psa", name="sp")
                for kc in range(8):
                    nc.tensor.matmul(
                        sp[:, 0:512],
                        lhsT=wts[kc][:, ts(t, 128)],
                        rhs=xts[kc][:, ts(c, 512)],
                        start=(kc == 0),
                        stop=(kc == 7),
                    )
                for kc in range(8):
                    nc.tensor.matmul(
                        sp[:, 512:1024],
                        lhsT=wts[kc][:, 256 + t * 128 : 256 + (t + 1) * 128],
                        rhs=xts[kc][:, ts(c, 512)],
                        start=(kc == 0),
                        stop=(kc == 7),
                    )
                qb = sb.tile([128, 1024], BF16, tag="qb", name="qb")
                nc.scalar.copy(qb[:], sp[:])
                wb = sb.tile([128, 1024], BF16, tag="wb", name="wb")
                nc.vector.tensor_copy(wb[0:32, :], qb[32:64, :])
                nc.vector.tensor_copy(wb[32:64, :], qb[0:32, :])
                nc.vector.tensor_copy(wb[64:96, :], qb[96:128, :])
                nc.vector.tensor_copy(wb[96:128, :], qb[64:96, :])
                t1 = sb.tile([128, 1024], BF16, tag="t1", name="t1")
                nc.vector.tensor_mul(t1[:, 0:512], qb[:, 0:512], ropec_sb[:, ts(c, 512)])
                nc.gpsimd.tensor_mul(t1[:, 512:1024], qb[:, 512:1024], ropec_sb[:, ts(c, 512)])
                t2 = sb.tile([128, 1024], BF16, tag="t2", name="t2")
                nc.vector.tensor_mul(t2[:, 0:512], wb[:, 0:512], ropes_sb[:, ts(c, 512)])
                nc.gpsimd.tensor_mul(t2[:, 512:1024], wb[:, 512:1024], ropes_sb[:, ts(c, 512)])
                nc.vector.tensor_add(
                    qf[t][:, ts(c, 512)], t1[:, 0:512], t2[:, 0:512]
                )
                nc.vector.tensor_add(
                    kf[t][:, ts(c, 512)], t1[:, 512:1024], t2[:, 512:1024]
                )

        # ---- attention: per (q-tile of 512, head-pair t), stream k-blocks ----
        for qi in range(4):
            for t in range(2):
                nblk = 4 * qi + 4
                oa = [
                    psB.tile([128, 512], F32, tag="psb", name=f"oa{_}")
                    for _ in range(2)
                ]
                for j in range(nblk):
                    dd = j - 4 * qi
                    nn = 512 if dd < 0 else 512 - 128 * dd
                    c0 = 512 - nn  # local q offset of the computed range
                    sp = psA.tile([128, 1024], F32, tag="psa", name="sp")
                    for hh in range(2):
                        r0 = 64 * hh
                        nc.tensor.matmul(
                            sp[:, hh * 512 : hh * 512 + nn],
                            lhsT=kf[t][r0 : r0 + 64, ts(j, 128)],
                            rhs=qf[t][r0 : r0 + 64, qi * 512 + c0 : (qi + 1) * 512],
                            start=True,
                            stop=True,
                        )
                    es = expp.tile([128, 1024], BF16, tag="es", name="es")
                    sp_v = sp[:].rearrange("p (b x) -> p b x", b=2)[:, :, 0:nn]
                    es_v = es[:].rearrange("p (b x) -> p b x", b=2)[:, :, 0:nn]
                    nc.scalar.activation(es_v, sp_v, Exp, scale=0.125)
                    if dd >= 0:
                        for hh in range(2):
                            # trimmed block: keep es[r, c] iff c >= r
                            nc.gpsimd.affine_select(
                                out=es[:, hh * 512 : hh * 512 + 128],
                                in_=es[:, hh * 512 : hh * 512 + 128],
                                compare_op=mybir.AluOpType.is_ge,
                                fill=0.0,
                                base=0,
                                pattern=[[1, 128]],
                                channel_multiplier=-1,
                            )
                    for hh in range(2):
                        h = 2 * t + hh
                        off = j * 260 + h * 65
                        nc.tensor.matmul(
                            oa[hh][:, c0:512],
                            lhsT=v_sb[:, off : off + 128],
                            rhs=es[:, hh * 512 : hh * 512 + nn],
                            start=(j == 0),
                            stop=(j == nblk - 1),
                        )
                for hh in range(2):
                    oasb = sb.tile([65, 512], BF16, tag="oasb", name="oasb")
                    nc.scalar.copy(oasb[:], oa[hh][0:65, :])
                    dps = psB.tile([64, 512], F32, tag="psb", name="dps")
                    nc.tensor.matmul(
                        dps[:],
                        lhsT=ones65[64:65, :],
                        rhs=oasb[64:65, :],
                        start=True,
                        stop=True,
                    )
                    rb = sb.tile([64, 512], F32, tag="rb", name="rb")
                    nc.vector.reciprocal_approx_fast(rb[:], dps[:])
                    nc.vector.tensor_mul(
                        ot[t][64 * hh : 64 * hh + 64, ts(qi, 512)],
                        oasb[0:64, :],
                        rb[:],
                    )

        # ---- output projection (partial over this core's 256 dims) ----
        for st in range(16):
            pp = psA.tile([128, 1024], F32, tag="psa", name="pp")
            for nh in range(2):
                for cc in range(2):
                    nc.tensor.matmul(
                        pp[:, ts(nh, 512)],
                        lhsT=ot[cc][:, ts(st, 128)],
                        rhs=wos[cc][:, ts(nh, 512)],
                        start=(cc == 0),
                        stop=(cc == 1),
                    )
            ob = outp.tile([128, 1024], BF16, tag="ob", name="ob")
            nc.vector.tensor_copy(ob[:], pp[:])
            nc.sync.dma_start(yp[ts(st, 128), :], ob[:])

    nc.compile()
    return nc


def _host_inputs(x, token_positions, Wq, Wk, Wv, Wo):
    x = np.asarray(x, dtype=np.float32)
    Wq = np.asarray(Wq, dtype=np.float32)
    Wk = np.asarray(Wk, dtype=np.float32)
    Wv = np.asarray(Wv, dtype=np.float32)
    Wo = np.asarray(Wo, dtype=np.float32)
    pos = np.asarray(token_positions).astype(np.float32)

    # rope tables, rows = [even(32) even(32) odd(32) odd(32)] freq index p%32
    f = np.arange(32, dtype=np.float32)
    inv = 1.0 / (THETA ** (2.0 * f / DK))
    ang = pos[:, None] * inv[None, :]  # [S, 32]
    cosT = np.cos(ang).T.astype(np.float32)  # [32, S]
    sinT = np.sin(ang).T.astype(np.float32)
    crow = np.tile(cosT, (4, 1))
    srow = np.concatenate([-sinT, sinT, -sinT, sinT], axis=0)

    ropec = np.ascontiguousarray(crow).astype(NPBF16)
    ropes = np.ascontiguousarray(srow).astype(NPBF16)

    ev = np.arange(0, DK, 2)
    od = np.arange(1, DK, 2)
    in_maps = []
    for core in range(8):
        bi, g = core // 4, core % 4
        xTb = np.ascontiguousarray(x[bi].T).astype(NPBF16)
        qk_idx = []
        for t in range(2):
            for hh, sel in ((2 * t, ev), (2 * t, od), (2 * t + 1, ev), (2 * t + 1, od)):
                qk_idx.append(DK * (4 * g + hh) + sel)
        qk_idx = np.concatenate(qk_idx)
        v_idx = 256 * g + np.arange(256)
        wq = Wq[qk_idx, :].T
        wk = Wk[qk_idx, :].T
        wv = Wv[v_idx, :].T
        wqkv = np.ascontiguousarray(
            np.concatenate([wq, wk, wv], axis=1)
        ).astype(NPBF16)
        woTl = np.ascontiguousarray(Wo[:, v_idx].T).astype(NPBF16)
        in_maps.append(
            dict(xT=xTb, wqkv=wqkv, woT=woTl, ropec=ropec, ropes=ropes)
        )
    return in_maps


def _run(inputs, trace=False, tmpdir=None):
    if "nc" not in _CACHE:
        _CACHE["nc"] = _build_nc()
    nc = _CACHE["nc"]
    in_maps = _host_inputs(**inputs)
    kw = {}
    if trace:
        kw = dict(trace=True, tmpdir=tmpdir)
    res = run_bass_kernel_spmd(nc, in_maps, list(range(8)), **kw)
    out = np.zeros((2, S, D), np.float32)
    for core in range(8):
        out[core // 4] += res.results[core]["yp"].astype(np.float32)
    return out, res


def kernel(**inputs):
    out, _ = _run(inputs, trace=False)
    return out

